# revision 12
# baseline (speedup 1.0000x reference)
"""Trainium2 Bass kernel for GQA attention prefill (B=2, T=2048, D=4096, N=32, K=8, H=128).

Sharding: 8 cores = 2 (batch) x 4 (head-groups). Each core handles one batch
element, 8 q-heads and its 2 kv-heads, producing a partial output projection
(summed over its heads). Host sums the 4 partials per batch element.

Per-core pipeline (PE matmuls in bf16/f32r -> fp32 PSUM):
  phase1: qT/kT [H,T] and v [S,H] projections from xT (streamed, batched
          DMAs), RoPE via a +-1 permutation matmul plus DVE muls with host
          cos/sin tables. Pass A computes k+v+q0,q1 so the k rope hides
          under pass B's q2..q7 matmuls.
  phase2: flash-style causal attention per (head, t-block 512): logitsT
          [s128,t512] = k-block @ qT (no transposes needed anywhere), diag
          masking via additive mask tiles, exp on ACT (scale=H^-0.5 folded
          in), AV and row-sum (ones-vector) matmuls accumulate in PSUM.
  phase3: y[t,d] += enc_h^T @ wo_h (bf16) accumulated over heads, merged DMAs.
"""

import os
import sys

import numpy as np

for _p in ("/opt/trn_rl_repo", "/root/.axon_site/_ro/trn_rl_repo"):
    if _p not in sys.path and os.path.isdir(_p):
        sys.path.append(_p)

import ml_dtypes

BF16 = ml_dtypes.bfloat16

P = 128
T = 2048
D = 4096
H = 128
NQ = 8   # q heads per core
NKV = 2  # kv heads per core
TB = 512
NTB = T // TB        # 4
DT = D // P          # 32 d-tiles
NSB = T // P         # 16 s-blocks
TC = TB // P         # 4 t-chunks per t-block
NDC = D // TB        # 8 d-chunks for the output projection
NEG = -1.0e38
SCALE = float(H) ** -0.5

_STATE = {}


def _build_nc():
    ablate = os.environ.get("KERNEL_ABLATE", "full")
    p3 = os.environ.get("KERNEL_P3", "0") == "1"
    nol = os.environ.get("KERNEL_NOL", "0") == "1"
    import concourse.mybir as mybir
    import concourse.tile as tile
    from concourse import bacc

    f32 = mybir.dt.float32
    f32r = mybir.dt.float32r
    bf16 = mybir.dt.bfloat16
    Alu = mybir.AluOpType
    Act = mybir.ActivationFunctionType

    nc = bacc.Bacc(None, target_bir_lowering=False, debug=False)

    xT = nc.dram_tensor("xT", [D, T], bf16, kind="ExternalInput")
    wq = nc.dram_tensor("wq", [D, NQ, H], bf16, kind="ExternalInput")
    wk = nc.dram_tensor("wk", [D, NKV, H], bf16, kind="ExternalInput")
    wv = nc.dram_tensor("wv", [D, NKV, H], bf16, kind="ExternalInput")
    wo = nc.dram_tensor("wo", [NQ, H, D], bf16, kind="ExternalInput")
    cos = nc.dram_tensor("cos", [P, T], bf16, kind="ExternalInput")
    sin = nc.dram_tensor("sin", [P, T], bf16, kind="ExternalInput")
    mrot = nc.dram_tensor("mrot", [P, P], f32r, kind="ExternalInput")
    masks = nc.dram_tensor("masks", [TC, P, TB], f32, kind="ExternalInput")
    ones = nc.dram_tensor("ones", [P, 1], f32r, kind="ExternalInput")
    y = nc.dram_tensor("y", [T, D], f32, kind="ExternalOutput")

    with tile.TileContext(nc) as tc:
        with (
            tc.tile_pool(name="const", bufs=1) as const,
            tc.tile_pool(name="wqp", bufs=2) as wqp,
            tc.tile_pool(name="xp", bufs=3) as xp,
            tc.tile_pool(name="qtp", bufs=2) as qtp,
            tc.tile_pool(name="rawp", bufs=4) as rawp,
            tc.tile_pool(name="ep", bufs=3) as ep,
            tc.tile_pool(name="encp", bufs=1) as encp,
            tc.tile_pool(name="lp", bufs=3) as lp,
            tc.tile_pool(name="wop", bufs=2) as wop,
            tc.tile_pool(name="yp", bufs=2) as yp,
            tc.tile_pool(name="ps", bufs=8, space="PSUM") as ps,
        ):
            # ---- resident constants (phase1 deps first) ----
            wk_sb = const.tile([P, DT, NKV, H], bf16, tag="wk")
            nc.sync.dma_start(wk_sb[:], wk.rearrange("(dt p) h e -> p dt h e", p=P))
            wv_sb = const.tile([P, DT, NKV * H], bf16, tag="wv")
            nc.sync.dma_start(wv_sb[:], wv.rearrange("(dt p) h e -> p dt (h e)", p=P))
            mrot_sb = const.tile([P, P], f32r, tag="mrot")
            nc.sync.dma_start(mrot_sb[:], mrot[:])
            cos_sb = const.tile([P, T], bf16, tag="cos")
            nc.sync.dma_start(cos_sb[:], cos[:])
            sin_sb = const.tile([P, T], bf16, tag="sin")
            nc.sync.dma_start(sin_sb[:], sin[:])
            mask_sb = const.tile([P, TC, TB], f32, tag="masks")
            nc.sync.dma_start(mask_sb[:], masks.rearrange("r p t -> p r t"))
            ones_sb = const.tile([P, 1], f32r, tag="ones")
            nc.sync.dma_start(ones_sb[:], ones[:])
            kT_all = const.tile([P, NKV, T], f32r, tag="kT")
            v_all = const.tile([P, NKV, NSB, H], f32r, tag="v")

            def rope(dst, src_ps, tb):
                """dst[:] = rope(src_ps) for one head's [H, TB] block."""
                cs = cos_sb[:, tb * TB:(tb + 1) * TB]
                sn = sin_sb[:, tb * TB:(tb + 1) * TB]
                raw = rawp.tile([P, TB], f32r, tag="raw")
                nc.scalar.copy(raw[:], src_ps[:])
                rot = ps.tile([P, TB], f32, tag="ps")
                nc.tensor.matmul(rot[:], mrot_sb[:], raw[:])
                tmp = rawp.tile([P, TB], f32, tag="raw")
                nc.vector.tensor_tensor(tmp[:], rot[:], sn, Alu.mult)
                nc.vector.tensor_tensor(dst, raw[:], cs, Alu.mult)
                nc.vector.tensor_tensor(dst, dst, tmp[:], Alu.add)

            def qkv_out(dst, src_ps, tb):
                if ablate == "p1g":
                    nc.scalar.copy(dst, src_ps[:])
                else:
                    rope(dst, src_ps, tb)

            for tb in range(NTB):
                tsl = slice(tb * TB, (tb + 1) * TB)

                # ---- phase 1 pass A: k(2) + v(4) + q0,q1 -> 8 psum banks ----
                psk = [ps.tile([P, TB], f32, tag="ps", name=f"psk{_k}")
                       for _k in range(NKV)]
                psv = [ps.tile([P, TB], f32, tag="ps", name=f"psv{_c}")
                       for _c in range(4)]
                psq = [ps.tile([P, TB], f32, tag="ps", name=f"psqA{_h}")
                       for _h in range(2)]
                for dt4 in range(DT // 4):
                    x4 = xp.tile([P, 4, TB], bf16, tag="xt")
                    nc.sync.dma_start(
                        x4[:], xT[dt4 * 4 * P:(dt4 + 1) * 4 * P, tsl]
                        .rearrange("(g p) t -> p g t", p=P))
                    w4 = wqp.tile([P, 4, 2, H], bf16, tag="wqA")
                    nc.sync.dma_start(
                        w4[:], wq[dt4 * 4 * P:(dt4 + 1) * 4 * P, 0:2, :]
                        .rearrange("(g p) h e -> p g h e", p=P))
                    for g in range(4):
                        dt = dt4 * 4 + g
                        st = dt == 0
                        sp = dt == DT - 1
                        for kk in range(NKV):
                            nc.tensor.matmul(psk[kk][:], wk_sb[:, dt, kk, :],
                                             x4[:, g, :], start=st, stop=sp)
                        for c in range(4):
                            nc.tensor.matmul(psv[c][:, 0:NKV * H],
                                             x4[:, g, c * P:(c + 1) * P],
                                             wv_sb[:, dt, :], start=st, stop=sp)
                        for h in range(2):
                            nc.tensor.matmul(psq[h][:], w4[:, g, h, :],
                                             x4[:, g, :], start=st, stop=sp)

                qt = qtp.tile([P, NQ, TB], f32r, tag="qt")
                for kk in range(NKV):
                    qkv_out(kT_all[:, kk, tsl], psk[kk], tb)
                for c in range(4):
                    nc.scalar.copy(
                        v_all[:, :, tb * TC + c, :],
                        psv[c][:, 0:NKV * H].rearrange("p (h e) -> p h e", h=NKV))
                for h in range(2):
                    qkv_out(qt[:, h, :], psq[h], tb)

                # ---- phase 1 pass B (and optional C): q2..q7 ----
                qpasses = [(2, 6, "wqB")] if not p3 else [(2, 4, "wqB"), (6, 2, "wqC")]
                for (h0, nh, wtag) in qpasses:
                    psq2 = [ps.tile([P, TB], f32, tag="ps", name=f"psqB{h0}_{_h}")
                            for _h in range(nh)]
                    for dt4 in range(DT // 4):
                        x4 = xp.tile([P, 4, TB], bf16, tag="xt")
                        nc.sync.dma_start(
                            x4[:], xT[dt4 * 4 * P:(dt4 + 1) * 4 * P, tsl]
                            .rearrange("(g p) t -> p g t", p=P))
                        w4 = wqp.tile([P, 4, nh, H], bf16, tag=wtag)
                        nc.sync.dma_start(
                            w4[:], wq[dt4 * 4 * P:(dt4 + 1) * 4 * P, h0:h0 + nh, :]
                            .rearrange("(g p) h e -> p g h e", p=P))
                        for g in range(4):
                            dt = dt4 * 4 + g
                            st = dt == 0
                            sp = dt == DT - 1
                            for h in range(nh):
                                nc.tensor.matmul(psq2[h][:], w4[:, g, h, :],
                                                 x4[:, g, :], start=st, stop=sp)
                    for h in range(nh):
                        qkv_out(qt[:, h0 + h, :], psq2[h], tb)

                if ablate in ("p1", "p1g"):
                    continue

                # ---- phase 2: attention for this t-block ----
                nsb = TC * (tb + 1)  # s-blocks participating (causal)
                enc = encp.tile([P, NQ, TB], bf16, tag="enc")
                for h in range(NQ):
                    kk = h // 4
                    enc_ps = ps.tile([P, TB], f32, tag="ps")
                    l_ps = None if nol else ps.tile([P, TB], f32, tag="ps")
                    for sb in range(nsb):
                        # diagonal blocks: only columns >= 128*r are unmasked
                        if sb >= nsb - TC:
                            r = sb - (nsb - TC)
                        else:
                            r = None
                        off = 0 if r is None else P * r
                        csl = slice(off, TB)
                        lg = ps.tile([P, TB], f32, tag="ps")
                        nc.tensor.matmul(
                            lg[:, csl], kT_all[:, kk, sb * P:(sb + 1) * P],
                            qt[:, h, csl])
                        if r is not None:
                            nc.vector.tensor_tensor(lg[:, csl], lg[:, csl],
                                                    mask_sb[:, r, csl], Alu.add)
                        ex = ep.tile([P, TB], f32r, tag="ex")
                        nc.scalar.activation(ex[:, csl], lg[:, csl], Act.Exp,
                                             scale=SCALE)
                        st = sb == 0
                        sp = sb == nsb - 1
                        nc.tensor.matmul(enc_ps[:, csl], v_all[:, kk, sb, :],
                                         ex[:, csl], start=st, stop=sp)
                        if not nol:
                            nc.tensor.matmul(l_ps[0:1, csl], ones_sb[:],
                                             ex[:, csl], start=st, stop=sp)
                    if nol:
                        nc.vector.tensor_copy(enc[:, h, :], enc_ps[:])
                    else:
                        rinv = lp.tile([1, TB], f32, tag="rinv")
                        nc.vector.reciprocal(rinv[:], l_ps[0:1, :])
                        rbc = lp.tile([P, TB], f32, tag="rbc")
                        nc.gpsimd.partition_broadcast(rbc[:], rinv[:])
                        nc.vector.tensor_tensor(enc[:, h, :], enc_ps[:], rbc[:],
                                                Alu.mult)

                if ablate == "p12":
                    continue

                # ---- phase 3: output projection for this t-block ----
                for dc in range(NDC):
                    wo8 = wop.tile([P, NQ, TB], bf16, tag="wo")
                    nc.sync.dma_start(
                        wo8[:], wo[:, :, dc * TB:(dc + 1) * TB]
                        .rearrange("h p d -> p h d"))
                    ys4 = yp.tile([P, TC, TB], f32, tag="ys")
                    for tc_i in range(TC):
                        yps = ps.tile([P, TB], f32, tag="ps")
                        for h in range(NQ):
                            nc.tensor.matmul(
                                yps[:],
                                enc[:, h, tc_i * P:(tc_i + 1) * P],
                                wo8[:, h, :],
                                start=(h == 0), stop=(h == NQ - 1))
                        nc.vector.tensor_copy(ys4[:, tc_i, :], yps[:])
                    nc.sync.dma_start(
                        y[tb * TB:(tb + 1) * TB, dc * TB:(dc + 1) * TB]
                        .rearrange("(tc p) d -> p tc d", p=P), ys4[:])

    nc.compile()
    return nc


def _get_nc():
    if "nc" not in _STATE:
        _STATE["nc"] = _build_nc()
    return _STATE["nc"]


def _make_in_maps(x, positions, wq, wkv, wo):
    """Build the 8 per-core input dicts (host-side sharding + tables)."""
    B = x.shape[0]
    in_maps = []

    tables = []
    for b in range(B):
        pos = np.asarray(positions[b], np.float64)
        timescale = 10000.0 ** ((2.0 / H) * np.arange(H // 2))
        rad = pos[:, None] / timescale[None, :]          # [T, H/2]
        c64 = np.cos(rad).T                              # [H/2, T]
        s64 = np.sin(rad).T
        tables.append((
            np.ascontiguousarray(np.concatenate([c64, c64], 0)).astype(BF16),
            np.ascontiguousarray(np.concatenate([s64, s64], 0)).astype(BF16),
        ))

    xTs = [np.ascontiguousarray(x[b].T).astype(BF16) for b in range(B)]

    M = np.zeros((P, P), np.float32)
    for h in range(H // 2):
        M[h, h + H // 2] = -1.0
        M[h + H // 2, h] = 1.0
    mrot = np.ascontiguousarray(M.T)

    msk = np.zeros((TC, P, TB), np.float32)
    for r in range(TC):
        i = np.arange(P)[:, None]
        j = np.arange(TB)[None, :]
        msk[r] = np.where(j >= i + P * r, 0.0, NEG)

    ones = np.ones((P, 1), np.float32)

    for c in range(8):
        b, hg = c // 4, c % 4
        qs = slice(NQ * hg, NQ * (hg + 1))
        ks = slice(NKV * hg, NKV * (hg + 1))
        cos_t, sin_t = tables[b]
        in_maps.append({
            "xT": xTs[b],
            "wq": np.ascontiguousarray(wq[qs].transpose(1, 0, 2)).astype(BF16),
            "wk": np.ascontiguousarray(wkv[0, ks].transpose(1, 0, 2)).astype(BF16),
            "wv": np.ascontiguousarray(wkv[1, ks].transpose(1, 0, 2)).astype(BF16),
            "wo": np.ascontiguousarray(wo[qs]).astype(BF16),
            "cos": cos_t,
            "sin": sin_t,
            "mrot": mrot,
            "masks": msk,
            "ones": ones,
        })
    return in_maps


def run_cores(in_maps, trace=False, trace_cores=None):
    from concourse.bass_utils import run_bass_kernel_spmd
    nc = _get_nc()
    kw = {}
    if trace:
        kw = dict(trace=True,
                  trace_cores=trace_cores or list(range(8)))
    return run_bass_kernel_spmd(nc, in_maps, core_ids=list(range(8)), **kw)


def kernel(**inputs):
    x = np.asarray(inputs["x"], np.float32)
    positions = np.asarray(inputs["positions"])
    wq = np.asarray(inputs["wq"], np.float32)
    wkv = np.asarray(inputs["wkv"], np.float32)
    wo = np.asarray(inputs["wo"], np.float32)
    B = x.shape[0]
    assert x.shape == (2, T, D) and wq.shape == (32, D, H)

    in_maps = _make_in_maps(x, positions, wq, wkv, wo)
    res = run_cores(in_maps)
    y = np.zeros((B, T, D), np.float32)
    for c, r in enumerate(res.results):
        y[c // 4] += r["y"]
    return y


if __name__ == "__main__":
    _build_nc()
    print("build OK")
